# revision 10
# baseline (speedup 1.0000x reference)
"""Trainium2 Bass kernel for nn_Aux2_46969762349381 (scatter_memory).

Computes, for embs [32, 2048, 1024] f32:
  status_probs = softmax(embs @ W_status.T + b_status)   # [B,T,5]
  flight_probs = softmax(embs @ W_flight.T + b_flight)   # [B,T,30]
  out = concat([s0, s2, s1, s4*flight, s3*flight], -1)   # [B,T,63]

Strategy (pure data parallel over batch, 8 cores; full inputs in, full
output out): each core owns 8192 tokens.

v2 design: the host pre-casts embs to bf16 and pre-transposes them into
the matmul-ready layout embsT[j*128+p, f] = embs[token(f), j*128+p] with
the token enumeration f = i*128 + q <-> t = q*64 + i chosen so the final
[tok, 63] output stores are >=2KB contiguous per partition. This removes
ALL on-device input transposes (the v1 kernel spent ~55us/core of PE time
PE-transposing embs tiles) and halves HBM read traffic (16MiB bf16 vs
32MiB f32 per core). The device is then a pure streaming pipeline:

  - DMA (HWDGE): per 1024-token batch, one 2MiB load embT [128, 8*1024]
    bf16, per-partition 2KB-contiguous runs -> near line rate.
  - PE: per 512-token group, 8 accumulating matmuls (lhsT = W chunk
    [128, 35] bf16, rhs = embT chunk [128, 512]) -> PSUM [35, 512] f32
    logits.T. Pure matmul stream keeps the HAM clock warm (2.4 GHz).
  - ACT: exp with the per-partition class bias fused, PSUM -> SBUF
    expT [35, 512] bf16.
  - PE: 4 transpose-mode matmuls per group flip expT back to token-major
    [128, 35] PSUM tiles (bf16: 1 cycle/row).
  - DVE: softmax normalization + outer-product scatter into
    o_sb [128, 8*63] f32, stored via >=2KB contiguous DMA.

Per-core budget: DMA in+out ~18MiB @ ~420GB/s ~ 45us; PE ~31us; DVE
~18us; ACT ~8us -> DMA-bound.

Precision: embs bf16, W bf16, exp output bf16, everything else f32.
Measured vs f32 reference (host numpy emulation): rel err ~2e-3.
"""

import os
import sys

import numpy as np

for _p in ("/opt/trn_rl_repo", "/root/.axon_site/_ro/trn_rl_repo"):
    if os.path.isdir(_p) and _p not in sys.path:
        sys.path.insert(0, _p)

from contextlib import ExitStack

import ml_dtypes

import concourse.bass as bass
import concourse.tile as tile
from concourse import mybir
from concourse.bass_utils import run_bass_kernel_spmd

N_CORES = 8
B, T, E = 32, 2048, 1024
NS, NF = 5, 30
NCLS = NS + NF          # 35 combined classes
OUTC = 63
P = 128                 # SBUF partitions
ECH = E // P            # 8 emb chunks of 128
GTOK = 512              # tokens per matmul group (PSUM bank limit: 512 f32)
GT = GTOK // P          # 4 token tiles per group
AG = 2                  # groups per assembly batch
NT = AG * GT            # 8 token tiles per assembly batch
BTOK = AG * GTOK        # 1024 tokens per assembly batch
XST = 36                # per-tile class stride in ps_xb (bf16: 72B, 4B-aligned)
F32 = mybir.dt.float32
BF16 = mybir.dt.bfloat16
EXP = mybir.ActivationFunctionType.Exp

BF16_NP = ml_dtypes.bfloat16


_CTRL_INSTS = ("InstDrain", "InstNoOp", "InstEventSemaphore",
               "InstUnconditionalBranch", "InstCompareAndBranch", "InstISA")


def _split_multiwait(nc, max_waits=1):
    """Workaround for this walrus build rejecting more than one sem-wait per
    instruction: move extra waits onto single-wait NoOps just before the
    instruction."""
    for bb in nc.m.functions[0].blocks:
        insts = list(bb.instructions)
        new_list = []
        changed = False
        for inst in insts:
            si = inst.sync_info
            cap = 1 if type(inst).__name__ in _CTRL_INSTS else max_waits
            if si is not None and si.on_wait and len(si.on_wait) > cap:
                waits = list(si.on_wait)
                for w in waits[:-cap]:
                    nop = mybir.InstNoOp(
                        name=nc.get_next_instruction_name(),
                        ins=[],
                        outs=[],
                        engine=inst.engine,
                        sync_info=mybir.SyncInfo(on_wait=[w], on_update=[]),
                    )
                    nc.register_instruction(nop)
                    new_list.append(nop)
                    changed = True
                inst.sync_info = mybir.SyncInfo(
                    on_wait=waits[-cap:], on_update=list(si.on_update)
                )
            new_list.append(inst)
        if changed:
            bb.instructions = new_list


def build_program(tok, passes=1, emb_bufs=4, out_batches=2, exp_dt=BF16,
                  ablate=()):
    """Per-core Bass program for `tok` tokens (tok % BTOK == 0).

    passes > 1 unrolls the whole pipeline that many times over the same
    data (idempotent) — benchmarking only.
    ablate: subset of {"out", "pe", "dve", "act"} — drop that part of the
    pipeline (timing ablations only; output is garbage).
    """
    # pipeline-suffix ablations for timing decomposition: each level also
    # drops everything after it (avoids uninitialized-read hazards)
    LV = {"nomm": 1, "noact": 2, "notr": 3, "nodve": 4, "noout": 5, "full": 6}
    lv = LV[ablate] if ablate else 6

    n_groups = tok // GTOK
    n_batches = n_groups // AG

    nc = bass.Bass("TRN2", num_devices=N_CORES)
    embsT_d = nc.dram_tensor("embsT", [E, tok], BF16, kind="ExternalInput")
    w_d = nc.dram_tensor("wt", [P, ECH * NCLS], BF16, kind="ExternalInput")
    b_d = nc.dram_tensor("bias", [NCLS, 1], F32, kind="ExternalInput")
    id_d = nc.dram_tensor("ident", [NCLS, NCLS], F32, kind="ExternalInput")
    out_d = nc.dram_tensor("out", [tok, OUTC], F32, kind="ExternalOutput")

    with tile.TileContext(nc) as tc, ExitStack() as ctx:
        consts = ctx.enter_context(tc.tile_pool(name="consts", bufs=1))
        emb_pool = ctx.enter_context(tc.tile_pool(name="emb", bufs=emb_bufs))
        expT_pool = ctx.enter_context(tc.tile_pool(name="expT", bufs=4))
        small = ctx.enter_context(tc.tile_pool(name="small", bufs=2))
        outsb = ctx.enter_context(tc.tile_pool(name="outsb", bufs=3))
        psmm_pool = ctx.enter_context(tc.tile_pool(name="psmm", bufs=4, space="PSUM"))
        psxb_pool = ctx.enter_context(tc.tile_pool(name="psxb", bufs=2, space="PSUM"))

        w_sb = consts.tile([P, ECH * NCLS], BF16)
        nc.sync.dma_start(w_sb[:], w_d.ap())
        b_sb = consts.tile([NCLS, 1], F32)
        nc.sync.dma_start(b_sb[:], b_d.ap())
        id_f32 = consts.tile([NCLS, NCLS], F32)
        nc.sync.dma_start(id_f32[:], id_d.ap())
        if exp_dt is BF16:
            id_sb = consts.tile([NCLS, NCLS], BF16)
            nc.vector.tensor_copy(id_sb[:], id_f32[:])
        else:
            id_sb = id_f32

        # Trigger the ACT exp table load (~2.7us) so it overlaps the first
        # embs DMA instead of stalling the first real exp.
        warm = consts.tile([NCLS, 1], F32)
        nc.scalar.activation(warm[:], b_sb[:], EXP)

        embsT_v = embsT_d.ap().rearrange("(j p) f -> p j f", p=P)
        out_v = out_d.ap().rearrange("(q i) c -> q i c", q=P)

        o_hold = None
        for ab0 in range(n_batches * passes):
            ab = ab0 % n_batches
            embT = emb_pool.tile([P, ECH * BTOK], BF16)
            embT_v = embT[:].rearrange("p (j t) -> p j t", t=BTOK)
            nc.sync.dma_start(embT_v, embsT_v[:, :, ab * BTOK:(ab + 1) * BTOK])

            ps_xb = psxb_pool.tile([P, NT * XST], exp_dt)
            for g2 in range(AG):
                if lv < 2:
                    continue
                ps_mm = psmm_pool.tile([NCLS, GTOK], F32)
                for j in range(ECH):
                    nc.tensor.matmul(
                        ps_mm[:],
                        w_sb[:, j * NCLS:(j + 1) * NCLS],
                        embT_v[:, j, g2 * GTOK:(g2 + 1) * GTOK],
                        start=(j == 0),
                        stop=(j == ECH - 1),
                    )
                if lv < 3:
                    continue
                expT = expT_pool.tile([NCLS, GTOK], exp_dt)
                nc.scalar.activation(expT[:], ps_mm[:], EXP, bias=b_sb[:], scale=1.0)
                if lv < 4:
                    continue
                for c in range(GT):
                    it = g2 * GT + c
                    nc.tensor.matmul(
                        ps_xb[:, it * XST:it * XST + NCLS],
                        expT[:, c * P:(c + 1) * P],
                        id_sb[:],
                        is_transpose=True,
                    )
            if lv < 5:
                continue

            # ---- softmax-normalize + scatter for NT tiles ----
            X = ps_xb[:].rearrange("p (i c) -> p i c", c=XST)  # [128, NT, 36] (35 used)
            sums = small.tile([P, 2 * NT], F32)
            nc.vector.reduce_sum(sums[:, 0:NT], X[:, :, 0:NS], axis=mybir.AxisListType.X)
            nc.vector.reduce_sum(
                sums[:, NT:2 * NT], X[:, :, NS:NCLS], axis=mybir.AxisListType.X
            )
            inv = small.tile([P, 2 * NT], F32)
            nc.vector.reciprocal(inv[:], sums[:])
            inv_s = inv[:, 0:NT]
            inv_f = inv[:, NT:2 * NT]
            dd = small.tile([P, NT], F32)
            nc.vector.tensor_mul(dd[:], inv_s, inv_f)
            cc = small.tile([P, 2 * NT], F32)
            nc.vector.tensor_mul(cc[:, 0:NT], X[:, :, 4], dd[:])  # book scale
            nc.vector.tensor_mul(cc[:, NT:2 * NT], X[:, :, 3], dd[:])  # change
            if ab % out_batches == 0:
                o_hold = outsb.tile([P, out_batches * NT * OUTC], F32)
            o_sb = o_hold[:, (ab % out_batches) * NT * OUTC:
                          (ab % out_batches + 1) * NT * OUTC]
            O = o_sb.rearrange("p (i c) -> p i c", c=OUTC)
            inv_s3 = inv_s.unsqueeze(2)
            nc.vector.tensor_mul(O[:, :, 0:1], X[:, :, 0:1], inv_s3)
            nc.vector.tensor_mul(O[:, :, 1:2], X[:, :, 2:3], inv_s3)
            nc.vector.tensor_mul(O[:, :, 2:3], X[:, :, 1:2], inv_s3)
            nc.vector.tensor_mul(
                O[:, :, 3:3 + NF],
                X[:, :, NS:NCLS],
                cc[:, 0:NT].unsqueeze(2).broadcast_to((P, NT, NF)),
            )
            nc.vector.tensor_mul(
                O[:, :, 3 + NF:OUTC],
                X[:, :, NS:NCLS],
                cc[:, NT:2 * NT].unsqueeze(2).broadcast_to((P, NT, NF)),
            )
            if ab % out_batches == out_batches - 1 and lv >= 6:
                lo = (ab - out_batches + 1) * NT
                nc.sync.dma_start(
                    out_v[:, lo:(ab + 1) * NT, :],
                    o_hold[:].rearrange("p (i c) -> p i c", c=OUTC),
                )

    _split_multiwait(nc)
    return nc


def host_inputs(W_status, b_status, W_flight, b_flight):
    W = np.concatenate([np.asarray(W_status), np.asarray(W_flight)], axis=0)
    W = np.ascontiguousarray(W, dtype=np.float32)          # [35, 1024]
    # w_host[p, j*35 + c] = W[c, j*128 + p]
    w_host = np.ascontiguousarray(
        W.T.reshape(ECH, P, NCLS).transpose(1, 0, 2).reshape(P, ECH * NCLS)
    ).astype(BF16_NP)
    b_host = np.ascontiguousarray(
        np.concatenate([np.asarray(b_status), np.asarray(b_flight)]).reshape(NCLS, 1),
        dtype=np.float32,
    )
    ident = np.eye(NCLS, dtype=np.float32)
    return w_host, b_host, ident


def prep_embsT(embs):
    """[B, T, E] f32 -> [N_CORES, E, tok] bf16 with f = i*128 + q <-> t = q*64 + i."""
    tok = B * T // N_CORES
    x = np.asarray(embs).reshape(N_CORES, P, tok // P, E)   # [core, q, i, e]
    x = x.astype(BF16_NP)
    x = x.transpose(0, 3, 2, 1)                              # [core, e, i, q]
    return np.ascontiguousarray(x).reshape(N_CORES, E, tok)


_program_cache = {}


def kernel(embs, W_status, b_status, W_flight, b_flight, **run_kwargs):
    tok = B * T // N_CORES
    embsT = prep_embsT(embs)
    w_host, b_host, ident = host_inputs(W_status, b_status, W_flight, b_flight)

    nc = _program_cache.get(tok)
    if nc is None:
        nc = build_program(tok)
        _program_cache[tok] = nc

    in_maps = [
        {"embsT": embsT[c], "wt": w_host, "bias": b_host, "ident": ident}
        for c in range(N_CORES)
    ]
    res = run_bass_kernel_spmd(
        nc, in_maps, core_ids=list(range(N_CORES)), **run_kwargs
    )
    out = np.concatenate([res.results[c]["out"] for c in range(N_CORES)], axis=0)
    out = out.reshape(B, T, OUTC)
    if run_kwargs:
        return out, res
    return out
